# revision 18
# baseline (speedup 1.0000x reference)
"""Exphydro (snow + 2-bucket soil + gamma-UH routing) Trainium2 Bass kernel.

Contract: kernel(x_phy [1095,15000,3] f32, params [15000,16] f32) -> [1095,15000] f32.
Shards the grid dim across 8 NeuronCores (1875 -> padded 1920 per core).

Per-core pipeline (single NEFF, fully unrolled):
  phase A  per time-block: forcing streams (rain/snow on DVE, freeze-pot
           prep on Pool, melt-pot / ln / exp / scaled-pet on ScalarE) in
           [g-partition, t-free] layout with per-partition param scalars.
  phase B  sequential scan over time: 23 DVE ops (incl. 2 custom fused DVE
           ops) + 6 GpSimd ops per step on [128 part, 15 chunk] tiles with
           strided stream access; soil bucket tracked in V=vad/vm units.
  phase C  15-tap x 2 gamma-UH causal conv as diagonal matmuls on TensorE
           accumulating in PSUM, interleaved under the scan (weights
           pre-scaled by vm; lgamma cancels in the softmax normalization).
"""
import numpy as np

T = 1095
TB = 128
NB = 9
TPAD = TB * NB           # 1152
G = 15000
NCORES = 8
GC = 1920                # padded grid per core
NCH = 15                 # chunks of 128 per core
L = 15                   # UH length
NZ = 1e-5

_CACHE = {}
_OPS = {}


def _ensure_custom_ops():
    if _OPS:
        return
    import concourse.dve_ops as dvo
    from concourse.dve_spec import Spec, Src0, Src1, relu, sq, lower, _has_src1
    from concourse.dve_uop import DveOpSpec

    def mk(name, body, ref):
        for o in dvo.OPS:
            if o.name == name:
                return o
        spec = Spec(body=body, reference=ref)
        op = dvo.DveOp(name, spec, subdim=False, uops_sha={})
        dvo.OPS.append(op)
        dvo._SUB_OPCODE_FOR_NAME[name] = dvo._CUSTOM_DVE_ROW_BASE + len(dvo.OPS) - 1
        dvo.CUSTOM_DVE_SPECS[name] = spec
        assert dvo._SUB_OPCODE_FOR_NAME[name] < 0x20
        for ver in ("v3", "v4"):
            s = DveOpSpec(name=name, opcode=dvo.get_dve_sub_opcode(name),
                          uops=lower(spec, ver=ver), rd1_en=_has_src1(spec))
            op.uops_sha[ver] = s.sha(ver)
        return op

    _OPS["subrelu"] = mk("XSUBRELU", relu(Src0 - Src1),
                         lambda in0, in1: np.maximum(in0 - in1, 0))
    _OPS["mulsq"] = mk("XMULSQ", Src1 * sq(Src0),
                       lambda in0, in1: in1 * in0 * in0)


def _3d(ap):
    return ap.rearrange("p (x n) -> p x n", x=1)


def _build_program():
    import concourse.bass as bass
    import concourse.mybir as mybir
    from concourse.tile import TileContext

    _ensure_custom_ops()

    dt = mybir.dt
    f32 = dt.float32
    Alu = mybir.AluOpType
    Act = mybir.ActivationFunctionType

    nc = bass.Bass()

    x_t = nc.dram_tensor("x", [3, TPAD, NCH, 128], f32, kind="ExternalInput")
    pr_t = nc.dram_tensor("pr", [16, NCH, 128], f32, kind="ExternalInput")
    out_t = nc.dram_tensor("out", [NCH, 128, T], f32, kind="ExternalOutput")

    # constants embedded in the NEFF
    ident_np = np.eye(128, dtype=np.float32)
    tl = np.arange(L, dtype=np.float32) + 0.5
    tl_np = np.tile(tl, (128, 1))
    lntl_np = np.tile(np.log(tl), (128, 1))
    ident_t = nc.inline_tensor(ident_np, "ident")
    tl_t = nc.inline_tensor(tl_np, "tlc")
    lntl_t = nc.inline_tensor(lntl_np, "lntlc")

    SERW = 16 + TPAD
    ser_ho = nc.dram_tensor("ser_ho", [NCH, 128, SERW], f32, kind="Internal")
    ser_qd = nc.dram_tensor("ser_qd", [NCH, 128, SERW], f32, kind="Internal")

    with TileContext(nc) as tc:
        with (
            tc.tile_pool(name="pers", bufs=1) as pers,
            tc.tile_pool(name="blk", bufs=1) as blk,
            tc.tile_pool(name="conv", bufs=3) as convp,
            tc.tile_pool(name="psum", bufs=2, space="PSUM") as psump,
        ):
            praw = pers.tile([128, 16 * NCH], f32, tag="praw", name="praw")
            # derived params, each [128, NCH]
            NPAR = 20
            pd = pers.tile([128, NPAR * NCH], f32, tag="pd", name="pd")
            (P_DDF, P_TBM, P_WRF, P_TBF, P_KF, P_FE, P_ETV, P_CR, P_CV,
             P_C2P, P_CP, P_VM, P_IVM, P_EPSV, P_DPHI, P_A1M, P_IB1,
             P_A2M, P_NDT, P_LNKF) = range(NPAR)
            pd2 = pers.tile([128, 2 * NCH], f32, tag="pd2", name="pd2")
            P2_IB2 = 0

            def pcol(j, c=None):
                if c is None:
                    return pd[:, j * NCH:(j + 1) * NCH]
                return pd[:, j * NCH + c:j * NCH + c + 1]

            def p2col(j, c=None):
                if c is None:
                    return pd2[:, j * NCH:(j + 1) * NCH]
                return pd2[:, j * NCH + c:j * NCH + c + 1]

            ident = pers.tile([128, 128], f32, tag="ident", name="identt")
            tlt = pers.tile([128, L], f32, tag="tlt", name="tlt")
            lntlt = pers.tile([128, L], f32, tag="lntlt", name="lntlt")
            uhw = pers.tile([128, 2 * NCH * L], f32, tag="uhw", name="uhw")  # W1 | W2
            st = pers.tile([128, 5 * NCH], f32, tag="st", name="stt_")  # Om | V | Sg | Phi0 | Phi1
            # DVE scratch: slots 0..22 (incl. parity-doubled producer slots);
            # Pool scratch: slots 23..26
            sc = pers.tile([128, 27 * NCH], f32, tag="sc", name="sc")
            zeros16 = pers.tile([128, 16], f32, tag="z16", name="z16")

            raw = [blk.tile([128, 3 * NCH * TB], f32, tag=f"raw{i}", name=f"raw{i}") for i in range(2)]
            strm = [blk.tile([128, 5 * NCH * TB], f32, tag=f"strm{i}", name=f"strm{i}") for i in range(2)]
            ser = [blk.tile([128, 2 * NCH * TB], f32, tag=f"ser{i}", name=f"ser{i}") for i in range(2)]
            pa_a = [blk.tile([128, TB], f32, tag=f"pa_a{i}", name=f"pa_a{i}") for i in range(2)]
            pa_b = [blk.tile([128, TB], f32, tag=f"pa_b{i}", name=f"pa_b{i}") for i in range(2)]
            pa_c = blk.tile([128, TB], f32, tag="pa_c", name="pa_c")

            nc.sync.dma_start(ident[:], ident_t[:, :])
            nc.sync.dma_start(tlt[:], tl_t[:, :])
            nc.sync.dma_start(lntlt[:], lntl_t[:, :])
            nc.sync.dma_start(praw[:], pr_t.rearrange("j c p -> p (j c)"))
            nc.vector.memset(zeros16[:], 0.0)

            def rawp(j):
                return praw[:, j * NCH:(j + 1) * NCH]

            ts = nc.vector.tensor_scalar
            tt = nc.vector.tensor_tensor
            stt = nc.vector.scalar_tensor_tensor
            ptt = nc.gpsimd.tensor_tensor
            pts = nc.gpsimd.tensor_scalar
            cdve = nc.vector._custom_dve

            # ---- derive params ----
            def ds(dst, j, lo, hi):
                ts(dst, rawp(j), float(hi - lo), float(lo), Alu.mult, Alu.add)

            ds(pcol(P_DDF), 0, 0.0, 40.0)
            ds(pcol(P_TBM), 1, -2.0, 3.0)
            ds(pcol(P_WRF), 2, 0.0, 0.5)
            ds(pcol(P_TBF), 3, -5.0, 2.0)
            ds(pcol(P_KF), 4, 0.0, 5.0)
            ds(pcol(P_FE), 5, 0.0, 1.0)
            ds(pcol(P_ETV), 6, 0.0, 1.0)        # ET for now; *ivm below
            ds(pcol(P_CR), 7, 0.0, 1.0)
            ds(pcol(P_C2P), 8, 1e-5, 0.02)
            ds(pcol(P_CV), 9, 0.0, 0.1)
            ds(pcol(P_CP), 10, 1e-5, 0.01)
            ds(pcol(P_VM), 11, 1e-3, 500.0)
            ds(pcol(P_A1M), 12, 0.3, 20.0)
            ts(pcol(P_A1M), pcol(P_A1M), -1.0, None, Alu.add)   # alpha1 - 1
            ds(pcol(P_IB1), 13, 0.01, 5.0)
            ds(pcol(P_A2M), 14, 0.5, 13.0)
            ts(pcol(P_A2M), pcol(P_A2M), -1.0, None, Alu.add)
            ds(p2col(P2_IB2), 15, 0.15, 1.5)
            nc.vector.reciprocal(pcol(P_IVM), pcol(P_VM))
            nc.vector.reciprocal(pcol(P_IB1), pcol(P_IB1))
            nc.vector.reciprocal(p2col(P2_IB2), p2col(P2_IB2))
            tt(pcol(P_ETV), pcol(P_ETV), pcol(P_IVM), Alu.mult)
            ts(pcol(P_EPSV), pcol(P_IVM), NZ, None, Alu.mult)
            ts(pcol(P_DPHI), pcol(P_CP), -1.0, 1.0, Alu.mult, Alu.add)
            # -ddf*Tbm (bias for the melt-pot Relu) and clamped ln(Kf)
            tt(pcol(P_NDT), pcol(P_DDF), pcol(P_TBM), Alu.mult)
            ts(pcol(P_NDT), pcol(P_NDT), -1.0, None, Alu.mult)
            nc.scalar.activation(pcol(P_LNKF), pcol(P_KF), Act.Ln)
            ts(pcol(P_LNKF), pcol(P_LNKF), -80.0, None, Alu.max)

            # ---- UH weights (softmax over taps; lgamma cancels) ----
            lgt = blk.tile([128, L], f32, tag="lgt", name="lgt")
            et = blk.tile([128, L], f32, tag="et", name="et")
            ssum = blk.tile([128, 1], f32, tag="ssum", name="ssum")
            for ui, amj in enumerate([P_A1M, P_A2M]):
                for c in range(NCH):
                    am = pcol(amj, c)
                    ib = pcol(P_IB1, c) if ui == 0 else p2col(P2_IB2, c)
                    ts(lgt[:], lntlt[:], am, None, Alu.mult)
                    stt(lgt[:], tlt[:], ib, lgt[:], Alu.mult, Alu.subtract)
                    nc.scalar.activation(et[:], lgt[:], Act.Exp, scale=-1.0)
                    nc.vector.tensor_reduce(ssum[:], et[:], mybir.AxisListType.X, Alu.add)
                    nc.vector.reciprocal(ssum[:], ssum[:])
                    ts(et[:], et[:], ssum[:], None, Alu.mult)
                    wdst = uhw[:, (ui * NCH + c) * L:(ui * NCH + c) * L + L]
                    ts(wdst, et[:], pcol(P_VM, c), None, Alu.mult)

            # ---- zero the 16-col series prefixes ----
            for c in range(NCH):
                nc.sync.dma_start(ser_ho[c, :, 0:16], zeros16[:])
                nc.sync.dma_start(ser_qd[c, :, 0:16], zeros16[:])

            # ---- init states: Om=Sg=1e-5, V=Phi=1e-5/vm ----
            OM, VV, SG, PH0, PH1 = 0, NCH, 2 * NCH, 3 * NCH, 4 * NCH
            nc.vector.memset(st[:, OM:OM + NCH], NZ)
            nc.vector.memset(st[:, SG:SG + NCH], NZ)
            ts(st[:, VV:VV + NCH], pcol(P_IVM), NZ, None, Alu.mult)
            ts(st[:, PH0:PH0 + NCH], pcol(P_IVM), NZ, None, Alu.mult)
            ts(st[:, PH1:PH1 + NCH], pcol(P_IVM), NZ, None, Alu.mult)

            def S(i):
                return sc[:, i * NCH:(i + 1) * NCH]

            # conv emission (interleaved under later scan blocks)
            FBS = [(f0, min(512, T - f0)) for f0 in range(0, T, 512)]

            def emit_conv(fb_idx):
                f0, F = FBS[fb_idx]
                for c in range(NCH):
                    ps = psump.tile([128, F], f32, tag="ps", name="ps")
                    first = True
                    for si, serd in enumerate([ser_ho, ser_qd]):
                        rhs = convp.tile([128, F + 14], f32, tag="rhs", name="rhs")
                        nc.sync.dma_start(rhs[:], serd[c, :, 2 + f0:2 + f0 + F + 14])
                        for l in range(L):
                            dg = convp.tile([128, 128], f32, tag="dg", name="dg")
                            wcol = uhw[:, (si * NCH + c) * L + l:(si * NCH + c) * L + l + 1]
                            nc.scalar.activation(dg[:], ident[:], Act.Copy, scale=wcol)
                            nc.tensor.matmul(
                                ps[:, 0:F], dg[:], rhs[:, 14 - l:14 - l + F],
                                start=first, stop=(si == 1 and l == L - 1),
                            )
                            first = False
                    ot = convp.tile([128, F], f32, tag="ot", name="ot")
                    nc.scalar.copy(ot[:], ps[:, 0:F])
                    nc.sync.dma_start(out_t[c, :, f0:f0 + F], ot[:])

            conv_after = {}
            for fb, (f0, F) in enumerate(FBS):
                conv_after.setdefault(min((f0 + F + TB - 1) // TB - 1, NB - 1),
                                      []).append(fb)

            for b in range(NB):
                pa = b % 2
                t0 = b * TB
                rw, sm, se = raw[pa], strm[pa], ser[pa]
                for ch in range(3):
                    for c in range(NCH):
                        nc.sync.dma_start(
                            rw[:, (ch * NCH + c) * TB:(ch * NCH + c + 1) * TB],
                            x_t[ch, t0:t0 + TB, c, :].rearrange("t p -> p t"),
                        )

                def rch(ch, c):
                    return rw[:, (ch * NCH + c) * TB:(ch * NCH + c) * TB + TB]

                def sch(s, c):
                    return sm[:, (s * NCH + c) * TB:(s * NCH + c) * TB + TB]

                # ---- phase A: streams PF(0) RPv(1) S(2) R(3) MP(4) ----
                for c in range(NCH):
                    Pc, Tc, Ec = rch(0, c), rch(1, c), rch(2, c)
                    stt(sch(3, c), Tc, 0.0, Pc, Alu.is_ge, Alu.mult)   # rain (DVE)
                    stt(sch(2, c), Tc, 0.0, Pc, Alu.is_lt, Alu.mult)   # snow (DVE)
                    ts(pa_c[:], Tc, pcol(P_TBM, c), pcol(P_DDF, c), Alu.subtract, Alu.mult)
                    ts(sch(4, c), pa_c[:], 0.0, None, Alu.max)          # melt pot (DVE)
                    paa, pab = pa_a[c % 2], pa_b[c % 2]
                    pts(paa[:], Tc, pcol(P_TBF, c), -1.0, Alu.subtract, Alu.mult)  # Pool
                    pts(paa[:], paa[:], NZ, None, Alu.max)                         # Pool
                    nc.scalar.activation(pab[:], paa[:], Act.Ln)        # ACT
                    nc.scalar.activation(paa[:], pab[:], Act.Exp, scale=pcol(P_FE, c))
                    ts(sch(0, c), paa[:], pcol(P_KF, c), None, Alu.mult)  # freeze pot
                    nc.scalar.activation(sch(1, c), Ec, Act.Copy,       # pet/vm (ACT)
                                         scale=pcol(P_ETV, c))

                sm4 = sm[:].rearrange("p (s c t) -> p t (s c)", s=5, c=NCH, t=TB)
                se4 = se[:].rearrange("p (s c t) -> p t (s c)", s=2, c=NCH, t=TB)

                # ---- phase B: scan (DVE 23 ops + Pool 6 ops per step) ----
                for t in range(TB):
                    Stt = sm4[:, t, 2 * NCH:3 * NCH]
                    Rtt = sm4[:, t, 3 * NCH:4 * NCH]
                    MPt = sm4[:, t, 4 * NCH:5 * NCH]
                    PR2 = sm4[:, t, 0:2 * NCH]
                    par = t % 2
                    # parity-doubled producer slots read by Pool (17-24)
                    sH0, sH1, sH2, sOV = (17 + 4 * par, 18 + 4 * par,
                                          19 + 4 * par, 20 + 4 * par)

                    tt(sc[:, 0:2 * NCH], PR2, st[:, 0:2 * NCH], Alu.min)  # f | aet
                    f, a1v = S(0), S(1)
                    tt(S(2), st[:, OM:OM + NCH], f, Alu.subtract)          # Oma
                    tt(S(3), st[:, SG:SG + NCH], f, Alu.add)               # Sga
                    tt(S(4), S(3), Stt, Alu.add)                           # Sgs
                    tt(S(5), MPt, S(4), Alu.min)                           # m
                    tt(st[:, SG:SG + NCH], S(4), S(5), Alu.subtract)       # Sg'
                    tt(S(6), S(2), S(5), Alu.add)                          # wa
                    tt(S(7), S(6), Rtt, Alu.add)                           # w
                    tt(S(8), pcol(P_WRF), st[:, SG:SG + NCH], Alu.mult)    # ret
                    tt(S(2), S(7), S(8), Alu.subtract)                     # w-ret (Oma slot dead)
                    ts(S(9), S(2), 0.0, None, Alu.max)                     # avail
                    tt(st[:, OM:OM + NCH], S(7), S(8), Alu.min)            # Om'
                    tt(S(10), S(9), pcol(P_IVM), Alu.mult)                 # Av
                    V = st[:, VV:VV + NCH]
                    tt(S(11), pcol(P_CR), V, Alu.mult)                     # u
                    tt(S(sH0), S(11), S(10), Alu.mult)                     # h0
                    tt(S(12), S(10), S(sH0), Alu.subtract)                 # i1
                    tt(S(11), V, V, Alu.mult)                              # V^2 (u slot dead)
                    tt(S(sH1), pcol(P_C2P), S(11), Alu.mult)               # h1
                    tt(S(sH2), pcol(P_CV), V, Alu.mult)                    # h2
                    tt(S(13), V, S(12), Alu.add)                           # x1
                    tt(S(14), S(13), a1v, Alu.subtract)                    # x2
                    tt(S(15), S(14), S(sH1), Alu.subtract)                 # x3
                    tt(S(16), S(15), S(sH2), Alu.subtract)                 # Vp
                    ts(S(sOV), S(16), 1.0, 0.0, Alu.subtract, Alu.max)     # ovf
                    stt(st[:, VV:VV + NCH], S(16), 1.0, pcol(P_EPSV),
                        Alu.min, Alu.max)                                  # V'
                    # phreatic chain on DVE (Phi parity-doubled; Pool reads old)
                    PHr = PH0 if par == 0 else PH1
                    PHw = PH1 if par == 0 else PH0
                    PhiO = st[:, PHr:PHr + NCH]
                    tt(S(26), pcol(P_DPHI), PhiO, Alu.mult)                # p1
                    tt(S(26), S(26), S(sH1), Alu.add)                      # p2
                    tt(st[:, PHw:PHw + NCH], S(26), pcol(P_EPSV), Alu.max)  # Phi'
                    # ---- Pool: dead-end output ops (slot 25) ----
                    ptt(se4[:, t, 0:NCH], S(sH0), S(sOV), Alu.add)         # ho
                    ptt(S(25), pcol(P_CP), PhiO, Alu.mult)                 # h3v
                    ptt(se4[:, t, NCH:2 * NCH], S(sH2), S(25), Alu.add)    # qd

                for c in range(NCH):
                    nc.sync.dma_start(
                        ser_ho[c, :, 16 + t0:16 + t0 + TB],
                        se[:, c * TB:(c + 1) * TB],
                    )
                    nc.sync.dma_start(
                        ser_qd[c, :, 16 + t0:16 + t0 + TB],
                        se[:, (NCH + c) * TB:(NCH + c + 1) * TB],
                    )

                for fb in conv_after.get(b, []):
                    emit_conv(fb)

    _split_multi_waits(nc)
    return nc


def _split_multi_waits(nc):
    """This container's walrus codegen accepts at most ONE sync wait per
    instruction; Tile emits several.  Hoist the excess onto same-engine
    NoOp carriers inserted immediately before."""
    from bass_rust import InstNoOp, SyncInfo

    cnt = 0
    for f in nc.m.functions:
        for bb in f.blocks:
            out = []
            changed = False
            for ins in bb.instructions:
                si = ins.sync_info
                w = list(si.on_wait) if si is not None and si.on_wait else []
                if len(w) > 1:
                    for extra in w[:-1]:
                        cnt += 1
                        nop = InstNoOp(name=f"WQ-{cnt}", engine=ins.engine)
                        nop.sync_info = SyncInfo(on_wait=[extra], on_update=[])
                        out.append(nop)
                    si.on_wait = [w[-1]]
                    changed = True
                out.append(ins)
            if changed:
                bb.instructions = out


def _get_program():
    if "nc" not in _CACHE:
        _CACHE["nc"] = _build_program()
    return _CACHE["nc"]


def kernel(x_phy: np.ndarray, params: np.ndarray) -> np.ndarray:
    from concourse.bass_utils import run_bass_kernel_spmd

    nc = _get_program()

    x_phy = np.ascontiguousarray(x_phy, dtype=np.float32)
    params = np.ascontiguousarray(params, dtype=np.float32)

    GPAD = NCORES * GC
    xp = np.zeros((TPAD, GPAD, 3), np.float32)
    xp[:T, :G] = x_phy
    pp = np.full((GPAD, 16), 0.5, np.float32)
    pp[:G] = params

    in_maps = []
    for k in range(NCORES):
        g0 = k * GC
        xk = np.ascontiguousarray(
            xp[:, g0:g0 + GC].transpose(2, 0, 1).reshape(3, TPAD, NCH, 128))
        pk = np.ascontiguousarray(
            pp[g0:g0 + GC].reshape(NCH, 128, 16).transpose(2, 0, 1))
        in_maps.append({"x": xk, "pr": pk})

    res = run_bass_kernel_spmd(nc, in_maps, core_ids=list(range(NCORES)))

    out = np.empty((T, G), np.float32)
    for k in range(NCORES):
        o = res.results[k]["out"]            # [NCH,128,T]
        g0 = k * GC
        hi = min(g0 + GC, G)
        flat = o.transpose(2, 0, 1).reshape(T, GC)
        out[:, g0:hi] = flat[:, :hi - g0]
    return out


# revision 27
# speedup vs baseline: 23.3115x; 23.3115x over previous
"""Exphydro (snow + 2-bucket soil + gamma-UH routing) Trainium2 Bass kernel.

Contract: kernel(x_phy [1095,15000,3] f32, params [15000,16] f32) -> [1095,15000] f32.
Shards the grid dim across 8 NeuronCores (1875 -> padded 1920 per core).

Per-core pipeline (single NEFF, fully unrolled):
  phase A  per time-block: forcing streams (rain/snow on DVE, freeze-pot
           prep on Pool, melt-pot / ln / exp / scaled-pet on ScalarE) in
           [g-partition, t-free] layout with per-partition param scalars.
  phase B  sequential scan over time: 23 DVE ops (incl. 2 custom fused DVE
           ops) + 6 GpSimd ops per step on [128 part, 15 chunk] tiles with
           strided stream access; soil bucket tracked in V=vad/vm units.
  phase C  15-tap x 2 gamma-UH causal conv as diagonal matmuls on TensorE
           accumulating in PSUM, interleaved under the scan (weights
           pre-scaled by vm; lgamma cancels in the softmax normalization).
"""
import numpy as np

T = 1095
TB = 128
NB = 9
TPAD = TB * NB           # 1152
G = 15000
NCORES = 8
GC = 1920                # padded grid per core
NCH = 15                 # chunks of 128 per core
L = 15                   # UH length
NZ = 1e-5

_CACHE = {}


def _build_program():
    import concourse.bass as bass
    import concourse.mybir as mybir
    from concourse.tile import TileContext

    dt = mybir.dt
    f32 = dt.float32
    Alu = mybir.AluOpType
    Act = mybir.ActivationFunctionType

    nc = bass.Bass()

    x_t = nc.dram_tensor("x", [3, TPAD, NCH, 128], f32, kind="ExternalInput")
    pr_t = nc.dram_tensor("pr", [16, NCH, 128], f32, kind="ExternalInput")
    out_t = nc.dram_tensor("out", [NCH, 128, T], f32, kind="ExternalOutput")

    # constants embedded in the NEFF
    ident_np = np.eye(128, dtype=np.float32)
    tl = np.arange(L, dtype=np.float32) + 0.5
    tl_np = np.tile(tl, (128, 1))
    lntl_np = np.tile(np.log(tl), (128, 1))
    ident_t = nc.inline_tensor(ident_np, "ident")
    tl_t = nc.inline_tensor(tl_np, "tlc")
    lntl_t = nc.inline_tensor(lntl_np, "lntlc")

    SERW = 16 + TPAD + 16
    ser_ho = nc.dram_tensor("ser_ho", [NCH, 128, SERW], f32, kind="Internal")
    ser_qd = nc.dram_tensor("ser_qd", [NCH, 128, SERW], f32, kind="Internal")
    ser_ph = nc.dram_tensor("ser_ph", [NCH, 128, SERW], f32, kind="Internal")

    with TileContext(nc) as tc:
        with (
            tc.tile_pool(name="pers", bufs=1) as pers,
            tc.tile_pool(name="blk", bufs=1) as blk,
            tc.tile_pool(name="conv", bufs=3) as convp,
            tc.tile_pool(name="psum", bufs=2, space="PSUM") as psump,
        ):
            praw = pers.tile([128, 16 * NCH], f32, tag="praw", name="praw")
            # derived params, each [128, NCH]
            NPAR = 20
            pd = pers.tile([128, NPAR * NCH], f32, tag="pd", name="pd")
            (P_DDF, P_TBM, P_WRF, P_TBF, P_KF, P_FE, P_ETV, P_CR, P_CV,
             P_C2P, P_CP, P_VM, P_IVM, P_EPSV, P_DPHI, P_A1M, P_IB1,
             P_A2M, P_NDT, P_LNKF) = range(NPAR)
            pd2 = pers.tile([128, 2 * NCH], f32, tag="pd2", name="pd2")
            P2_IB2 = 0

            def pcol(j, c=None):
                if c is None:
                    return pd[:, j * NCH:(j + 1) * NCH]
                return pd[:, j * NCH + c:j * NCH + c + 1]

            def p2col(j, c=None):
                if c is None:
                    return pd2[:, j * NCH:(j + 1) * NCH]
                return pd2[:, j * NCH + c:j * NCH + c + 1]

            ident = pers.tile([128, 128], f32, tag="ident", name="identt")
            tlt = pers.tile([128, L], f32, tag="tlt", name="tlt")
            lntlt = pers.tile([128, L], f32, tag="lntlt", name="lntlt")
            uhw = pers.tile([128, 3 * NCH * L], f32, tag="uhw", name="uhw")  # W1 | W2h | W2p
            st = pers.tile([128, 4 * NCH], f32, tag="st", name="stt_")  # Om | Sg | V0 | V1
            # DVE scratch: slots 0..22 (incl. parity-doubled producer slots);
            # Pool scratch: slots 23..26
            sc = pers.tile([128, 27 * NCH], f32, tag="sc", name="sc")
            zeros16 = pers.tile([128, 16], f32, tag="z16", name="z16")

            raw = [blk.tile([128, 3 * NCH * TB], f32, tag="raw0", name="raw0")]
            strm = [blk.tile([128, 5 * NCH * TB], f32, tag=f"strm{i}", name=f"strm{i}") for i in range(2)]
            ser = [blk.tile([128, 4 * NCH * TB], f32, tag=f"ser{i}", name=f"ser{i}") for i in range(2)]
            pa_a = [blk.tile([128, TB], f32, tag=f"pa_a{i}", name=f"pa_a{i}") for i in range(2)]
            pa_b = [blk.tile([128, TB], f32, tag=f"pa_b{i}", name=f"pa_b{i}") for i in range(2)]
            pa_c = blk.tile([128, TB], f32, tag="pa_c", name="pa_c")

            nc.sync.dma_start(ident[:], ident_t[:, :])
            nc.sync.dma_start(tlt[:], tl_t[:, :])
            nc.sync.dma_start(lntlt[:], lntl_t[:, :])
            nc.sync.dma_start(praw[:], pr_t.rearrange("j c p -> p (j c)"))
            nc.vector.memset(zeros16[:], 0.0)

            def rawp(j):
                return praw[:, j * NCH:(j + 1) * NCH]

            ts = nc.vector.tensor_scalar
            tt = nc.vector.tensor_tensor
            stt = nc.vector.scalar_tensor_tensor
            ptt = nc.gpsimd.tensor_tensor
            pts = nc.gpsimd.tensor_scalar

            # ---- derive params ----
            def ds(dst, j, lo, hi):
                ts(dst, rawp(j), float(hi - lo), float(lo), Alu.mult, Alu.add)

            ds(pcol(P_DDF), 0, 0.0, 40.0)
            ds(pcol(P_TBM), 1, -2.0, 3.0)
            ds(pcol(P_WRF), 2, 0.0, 0.5)
            ds(pcol(P_TBF), 3, -5.0, 2.0)
            ds(pcol(P_KF), 4, 0.0, 5.0)
            ds(pcol(P_FE), 5, 0.0, 1.0)
            ds(pcol(P_ETV), 6, 0.0, 1.0)        # ET for now; *ivm below
            ds(pcol(P_CR), 7, 0.0, 1.0)
            ds(pcol(P_C2P), 8, 1e-5, 0.02)
            ds(pcol(P_CV), 9, 0.0, 0.1)
            ds(pcol(P_CP), 10, 1e-5, 0.01)
            ds(pcol(P_VM), 11, 1e-3, 500.0)
            ds(pcol(P_A1M), 12, 0.3, 20.0)
            ts(pcol(P_A1M), pcol(P_A1M), -1.0, None, Alu.add)   # alpha1 - 1
            ds(pcol(P_IB1), 13, 0.01, 5.0)
            ds(pcol(P_A2M), 14, 0.5, 13.0)
            ts(pcol(P_A2M), pcol(P_A2M), -1.0, None, Alu.add)
            ds(p2col(P2_IB2), 15, 0.15, 1.5)
            nc.vector.reciprocal(pcol(P_IVM), pcol(P_VM))
            nc.vector.reciprocal(pcol(P_IB1), pcol(P_IB1))
            nc.vector.reciprocal(p2col(P2_IB2), p2col(P2_IB2))
            tt(pcol(P_ETV), pcol(P_ETV), pcol(P_IVM), Alu.mult)
            ts(pcol(P_EPSV), pcol(P_IVM), NZ, None, Alu.mult)
            ts(pcol(P_DPHI), pcol(P_CP), -1.0, 1.0, Alu.mult, Alu.add)
            # -ddf*Tbm (bias for the melt-pot Relu) and clamped ln(Kf)
            tt(pcol(P_NDT), pcol(P_DDF), pcol(P_TBM), Alu.mult)
            ts(pcol(P_NDT), pcol(P_NDT), -1.0, None, Alu.mult)
            nc.scalar.activation(pcol(P_LNKF), pcol(P_KF), Act.Ln)
            ts(pcol(P_LNKF), pcol(P_LNKF), -80.0, None, Alu.max)

            # ---- UH weights (softmax over taps; lgamma cancels) ----
            lgt = blk.tile([128, L], f32, tag="lgt", name="lgt")
            et = blk.tile([128, L], f32, tag="et", name="et")
            ssum = blk.tile([128, 1], f32, tag="ssum", name="ssum")
            for ui, amj in enumerate([P_A1M, P_A2M]):
                for c in range(NCH):
                    am = pcol(amj, c)
                    ib = pcol(P_IB1, c) if ui == 0 else p2col(P2_IB2, c)
                    ts(lgt[:], lntlt[:], am, None, Alu.mult)
                    stt(lgt[:], tlt[:], ib, lgt[:], Alu.mult, Alu.subtract)
                    nc.scalar.activation(et[:], lgt[:], Act.Exp, scale=-1.0)
                    nc.vector.tensor_reduce(ssum[:], et[:], mybir.AxisListType.X, Alu.add)
                    nc.vector.reciprocal(ssum[:], ssum[:])
                    ts(et[:], et[:], ssum[:], None, Alu.mult)
                    wdst = uhw[:, (ui * NCH + c) * L:(ui * NCH + c) * L + L]
                    ts(wdst, et[:], pcol(P_VM, c), None, Alu.mult)
                    if ui == 1:
                        w2p = uhw[:, (2 * NCH + c) * L:(2 * NCH + c) * L + L]
                        ts(w2p, wdst, pcol(P_CP, c), None, Alu.mult)

            # ---- zero the 16-col series prefixes ----
            for c in range(NCH):
                nc.sync.dma_start(ser_ho[c, :, 0:16], zeros16[:])
                nc.sync.dma_start(ser_qd[c, :, 0:16], zeros16[:])
                nc.sync.dma_start(ser_ph[c, :, 0:16], zeros16[:])
                nc.sync.dma_start(ser_ph[c, :, 16:17], pcol(P_EPSV, c))

            # ---- init states: Om=Sg=1e-5, V=Phi=1e-5/vm ----
            OM, SG, V0, V1 = 0, NCH, 2 * NCH, 3 * NCH
            nc.vector.memset(st[:, OM:OM + NCH], NZ)
            nc.vector.memset(st[:, SG:SG + NCH], NZ)
            ts(st[:, V0:V0 + NCH], pcol(P_IVM), NZ, None, Alu.mult)
            ts(st[:, V1:V1 + NCH], pcol(P_IVM), NZ, None, Alu.mult)

            def S(i):
                return sc[:, i * NCH:(i + 1) * NCH]

            # conv emission (interleaved under later scan blocks)
            FBS = [(f0, min(512, T - f0)) for f0 in range(0, T, 512)]

            def emit_conv(fb_idx):
                f0, F = FBS[fb_idx]
                for c in range(NCH):
                    ps = psump.tile([128, F], f32, tag="ps", name="ps")
                    first = True
                    for si, serd in enumerate([ser_ho, ser_qd, ser_ph]):
                        rhs = convp.tile([128, F + 14], f32, tag="rhs", name="rhs")
                        nc.sync.dma_start(rhs[:], serd[c, :, 2 + f0:2 + f0 + F + 14])
                        for l in range(L):
                            dg = convp.tile([128, 128], f32, tag="dg", name="dg")
                            wcol = uhw[:, (si * NCH + c) * L + l:(si * NCH + c) * L + l + 1]
                            nc.scalar.activation(dg[:], ident[:], Act.Copy, scale=wcol)
                            nc.tensor.matmul(
                                ps[:, 0:F], dg[:], rhs[:, 14 - l:14 - l + F],
                                start=first, stop=(si == 2 and l == L - 1),
                            )
                            first = False
                    ot = convp.tile([128, F], f32, tag="ot", name="ot")
                    nc.scalar.copy(ot[:], ps[:, 0:F])
                    nc.sync.dma_start(out_t[c, :, f0:f0 + F], ot[:])

            conv_after = {}
            for fb, (f0, F) in enumerate(FBS):
                conv_after.setdefault(min((f0 + F + TB - 1) // TB - 1, NB - 1),
                                      []).append(fb)

            for b in range(NB):
                pa = b % 2
                t0 = b * TB
                rw, sm, se = raw[0], strm[pa], ser[pa]
                for ch in range(3):
                    for c in range(NCH):
                        nc.sync.dma_start(
                            rw[:, (ch * NCH + c) * TB:(ch * NCH + c + 1) * TB],
                            x_t[ch, t0:t0 + TB, c, :].rearrange("t p -> p t"),
                        )

                def rch(ch, c):
                    return rw[:, (ch * NCH + c) * TB:(ch * NCH + c) * TB + TB]

                def sch(s, c):
                    return sm[:, (s * NCH + c) * TB:(s * NCH + c) * TB + TB]

                # ---- phase A: streams PF(0) RPv(1) S(2) R(3) MP(4) ----
                for c in range(NCH):
                    Pc, Tc, Ec = rch(0, c), rch(1, c), rch(2, c)
                    stt(sch(3, c), Tc, 0.0, Pc, Alu.is_ge, Alu.mult)   # rain (DVE)
                    stt(sch(2, c), Tc, 0.0, Pc, Alu.is_lt, Alu.mult)   # snow (DVE)
                    ts(pa_c[:], Tc, pcol(P_TBM, c), pcol(P_DDF, c), Alu.subtract, Alu.mult)
                    ts(sch(4, c), pa_c[:], 0.0, None, Alu.max)          # melt pot (DVE)
                    paa, pab = pa_a[c % 2], pa_b[c % 2]
                    pts(paa[:], Tc, pcol(P_TBF, c), -1.0, Alu.subtract, Alu.mult)  # Pool
                    pts(paa[:], paa[:], NZ, None, Alu.max)                         # Pool
                    nc.scalar.activation(pab[:], paa[:], Act.Ln)        # ACT
                    nc.scalar.activation(paa[:], pab[:], Act.Exp, scale=pcol(P_FE, c))
                    ts(sch(0, c), paa[:], pcol(P_KF, c), None, Alu.mult)  # freeze pot
                    nc.scalar.activation(sch(1, c), Ec, Act.Copy,       # pet/vm (ACT)
                                         scale=pcol(P_ETV, c))

                sm4 = sm[:].rearrange("p (s c t) -> p t (s c)", s=5, c=NCH, t=TB)
                se4 = se[:].rearrange("p (s c t) -> p t (s c)", s=4, c=NCH, t=TB)
                sep4 = ser[1 - pa][:].rearrange("p (s c t) -> p t (s c)", s=4, c=NCH, t=TB)

                # ---- phase B: pipelined scan ----
                # DVE: snow chain + V-clip + aet (13 ops/step)
                # Pool: soil arithmetic chain (11 ops/step) writing the
                #       h0/h1/h2/Vp series slots consumed by batch-post.
                SL_H0, SL_H1, SL_H2, SL_VP = 0, NCH, 2 * NCH, 3 * NCH
                for t in range(TB):
                    k = t0 + t
                    PFt = sm4[:, t, 0:NCH]
                    RPt = sm4[:, t, NCH:2 * NCH]
                    Stt = sm4[:, t, 2 * NCH:3 * NCH]
                    Rtt = sm4[:, t, 3 * NCH:4 * NCH]
                    MPt = sm4[:, t, 4 * NCH:5 * NCH]
                    Vcur = st[:, V0 + (k % 2) * NCH:V0 + (k % 2) * NCH + NCH]
                    if t == 0:
                        VpPrev = sep4[:, TB - 1, SL_VP:SL_VP + NCH] if b > 0 else None
                    else:
                        VpPrev = se4[:, t - 1, SL_VP:SL_VP + NCH]

                    # -- DVE snow chain (iteration k) --
                    tt(S(0), PFt, st[:, OM:OM + NCH], Alu.min)             # f
                    tt(S(2), st[:, OM:OM + NCH], S(0), Alu.subtract)       # Oma
                    tt(S(3), st[:, SG:SG + NCH], S(0), Alu.add)            # Sga
                    tt(S(4), S(3), Stt, Alu.add)                           # Sgs
                    tt(S(5), MPt, S(4), Alu.min)                           # m
                    tt(st[:, SG:SG + NCH], S(4), S(5), Alu.subtract)       # Sg'
                    tt(S(6), S(2), S(5), Alu.add)                          # wa
                    tt(S(7), S(6), Rtt, Alu.add)                           # w
                    tt(S(8), pcol(P_WRF), st[:, SG:SG + NCH], Alu.mult)    # ret
                    tt(st[:, OM:OM + NCH], S(7), S(8), Alu.min)            # Om'
                    tt(S(9), S(7), st[:, OM:OM + NCH], Alu.subtract)       # avail = w - Om'
                    # V'(k-1): clip previous step's Vp into the V state slot
                    if VpPrev is not None:
                        stt(Vcur, VpPrev, 1.0, pcol(P_EPSV), Alu.min, Alu.max)
                    tt(S(1), RPt, Vcur, Alu.min)                           # aet
                    # -- Pool soil chain (iteration k) --
                    ptt(S(10), S(9), pcol(P_IVM), Alu.mult)                # Av
                    ptt(S(11), pcol(P_CR), Vcur, Alu.mult)                 # u
                    ptt(se4[:, t, SL_H0:SL_H0 + NCH], S(11), S(10), Alu.mult)   # h0
                    ptt(S(12), S(10), se4[:, t, SL_H0:SL_H0 + NCH], Alu.subtract)  # i1
                    ptt(S(13), Vcur, S(12), Alu.add)                       # x1
                    ptt(S(14), S(13), S(1), Alu.subtract)                  # x2
                    ptt(S(15), Vcur, Vcur, Alu.mult)                       # v2
                    ptt(se4[:, t, SL_H1:SL_H1 + NCH], pcol(P_C2P), S(15), Alu.mult)  # h1
                    ptt(se4[:, t, SL_H2:SL_H2 + NCH], pcol(P_CV), Vcur, Alu.mult)    # h2
                    ptt(S(16), S(14), se4[:, t, SL_H1:SL_H1 + NCH], Alu.subtract)    # x3
                    ptt(se4[:, t, SL_VP:SL_VP + NCH], S(16),
                        se4[:, t, SL_H2:SL_H2 + NCH], Alu.subtract)        # Vp

                # ---- batch-post: ho, phi (TTS), series DMA out ----
                seb = se[:]
                for c in range(NCH):
                    h0b = seb[:, c * TB:c * TB + TB]
                    h1b = seb[:, (NCH + c) * TB:(NCH + c) * TB + TB]
                    h2b = seb[:, (2 * NCH + c) * TB:(2 * NCH + c) * TB + TB]
                    vpb = seb[:, (3 * NCH + c) * TB:(3 * NCH + c) * TB + TB]
                    ts(pa_c[:], vpb, 1.0, 0.0, Alu.subtract, Alu.max)      # ovf
                    tt(h0b, h0b, pa_c[:], Alu.add)                         # ho (in-place)
                    if b == 0:
                        phi_init = pcol(P_EPSV, c)
                    else:
                        phi_init = ser[1 - pa][:, (NCH + c) * TB + TB - 1:
                                               (NCH + c) * TB + TB]
                    nc.vector.tensor_tensor_scan(
                        h1b, pcol(P_DPHI, c).broadcast_to((128, TB)), h1b,
                        phi_init, Alu.mult, Alu.add)                       # phi (in-place over h1)
                    ts(h1b, h1b, pcol(P_EPSV, c), None, Alu.max)           # eps floor
                    nc.sync.dma_start(ser_ho[c, :, 16 + t0:16 + t0 + TB], h0b)
                    nc.sync.dma_start(ser_qd[c, :, 16 + t0:16 + t0 + TB], h2b)
                    nc.sync.dma_start(ser_ph[c, :, 17 + t0:17 + t0 + TB], h1b)

                for fb in conv_after.get(b, []):
                    emit_conv(fb)

    _split_multi_waits(nc)
    return nc


def _split_multi_waits(nc):
    """This container's walrus codegen accepts at most ONE sync wait per
    instruction; Tile emits several.  Hoist the excess onto same-engine
    NoOp carriers inserted immediately before."""
    from bass_rust import InstNoOp, SyncInfo

    cnt = 0
    for f in nc.m.functions:
        for bb in f.blocks:
            out = []
            changed = False
            for ins in bb.instructions:
                si = ins.sync_info
                w = list(si.on_wait) if si is not None and si.on_wait else []
                if len(w) > 1:
                    for extra in w[:-1]:
                        cnt += 1
                        nop = InstNoOp(name=f"WQ-{cnt}", engine=ins.engine)
                        nop.sync_info = SyncInfo(on_wait=[extra], on_update=[])
                        out.append(nop)
                    si.on_wait = [w[-1]]
                    changed = True
                out.append(ins)
            if changed:
                bb.instructions = out


def _get_program():
    if "nc" not in _CACHE:
        _CACHE["nc"] = _build_program()
    return _CACHE["nc"]


def kernel(x_phy: np.ndarray, params: np.ndarray) -> np.ndarray:
    from concourse.bass_utils import run_bass_kernel_spmd

    nc = _get_program()

    x_phy = np.ascontiguousarray(x_phy, dtype=np.float32)
    params = np.ascontiguousarray(params, dtype=np.float32)

    GPAD = NCORES * GC
    xp = np.zeros((TPAD, GPAD, 3), np.float32)
    xp[:T, :G] = x_phy
    pp = np.full((GPAD, 16), 0.5, np.float32)
    pp[:G] = params

    in_maps = []
    for k in range(NCORES):
        g0 = k * GC
        xk = np.ascontiguousarray(
            xp[:, g0:g0 + GC].transpose(2, 0, 1).reshape(3, TPAD, NCH, 128))
        pk = np.ascontiguousarray(
            pp[g0:g0 + GC].reshape(NCH, 128, 16).transpose(2, 0, 1))
        in_maps.append({"x": xk, "pr": pk})

    res = run_bass_kernel_spmd(nc, in_maps, core_ids=list(range(NCORES)))

    out = np.empty((T, G), np.float32)
    for k in range(NCORES):
        o = res.results[k]["out"]            # [NCH,128,T]
        g0 = k * GC
        hi = min(g0 + GC, G)
        flat = o.transpose(2, 0, 1).reshape(T, GC)
        out[:, g0:hi] = flat[:, :hi - g0]
    return out
